# revision 4
# baseline (speedup 1.0000x reference)
"""Trainium2 Bass kernel for MultiHeadAttentionWithRope.

Problem: B=2, T=2048, C=2048, H=16 heads, D=128 head_dim, fp32 I/O.
  qkv = x @ W_qkv; q,k -> RoPE (adjacent-pair, torchtune) -> causal SDPA
  -> out = o @ W_out

Sharding (8 cores): 2 batches x 4 head-groups (4 heads each).
Each core computes a partial output out_partial[b] = o_heads @ W_out_rows
(bf16); the host sums the 4 partials per batch in f32.

Per-core layout trick: everything is computed in "transposed" space.
  - host pre-transposes x[b] -> xT [C, T] (bf16)
  - qT,kT = (W_q|k)^T x in [D, T] layout directly (lhsT = W slice as stored)
  - RoPE dims are de-interleaved by permuting W_qkv q/k columns on the host
    (even dims first). S = q.k is invariant under a shared permutation.
  - scores are computed transposed: S.T[k,q] = matmul(lhsT=kT, rhs=qT),
    so softmax probs P~[k,q] feed the PV matmul with no on-device transpose.
  - no max-subtraction in softmax (logits are bounded: |S|<~6 at this scale)
  - row-sums l: probs chunks are pairwise tree-summed in bf16 on DVE/Pool,
    then ONE ones-lhsT matmul per (qt,h) reduces the accumulated [128,512]
    tile over partitions (vs. a rowsum matmul per k-chunk: ~25us less PE).
  - 1/l via DVE reciprocal (frees the ACT engine, which exp saturates);
    1/l folded into oT before the output projection.
All matmuls in bf16 (fp32 accumulate in PSUM).

Startup: per-chunk x/w tiles (fine-grained DMA deps) and QKV matmuls
emitted in groups of 4 heads kc-outer, so the PE consumes chunks at DMA
arrival pace instead of waiting for whole tensors.
"""

import sys

sys.path.insert(0, "/opt/trn_rl_repo")

import numpy as np
import ml_dtypes

import concourse.bass as bass
import concourse.tile as tile
from concourse import mybir
from concourse.bass import ts
from concourse.bass_utils import run_bass_kernel_spmd

# Provide antenv.axon_hooks (absent in this container) so trace=True can use
# the axon NTFF profiling path.
def _ensure_axon_hooks():
    import types

    try:
        from antenv import axon_hooks  # noqa: F401
        return
    except ImportError:
        pass
    import antenv

    mod = types.ModuleType("antenv.axon_hooks")
    mod._hook = None

    def set_axon_ntff_profile_hook(h):
        mod._hook = h

    def get_axon_ntff_profile_hook():
        return mod._hook

    mod.set_axon_ntff_profile_hook = set_axon_ntff_profile_hook
    mod.get_axon_ntff_profile_hook = get_axon_ntff_profile_hook
    sys.modules["antenv.axon_hooks"] = mod
    antenv.axon_hooks = mod
    try:
        from trn_agent_boot.trn_boot import _ntff_profile_via_ctypes

        hook = _ntff_profile_via_ctypes("/opt/axon/libaxon_pjrt.so")
        if hook is not None:
            mod._hook = hook
    except Exception:
        pass


_ensure_axon_hooks()

# ---------------------------------------------------------------------------
# This walrus build supports only ONE sync-wait command per instruction.
# TileContext's sem assignment can attach several waits to one instruction
# (and its exit drain aggregates many). Post-pass: hoist excess waits onto
# same-engine NoOps inserted immediately before the instruction -- the
# engine blocks on each wait in order, so semantics are identical.
MAX_WAITS_PER_INST = 1


_ALL_ENGINES = [
    mybir.EngineType.PE,
    mybir.EngineType.Activation,
    mybir.EngineType.DVE,
    mybir.EngineType.Pool,
    mybir.EngineType.SP,
]


def _split_sync_waits(nc):
    for f in nc.m.functions:
        for blk in f.blocks:
            new_insts = []
            for ins in blk.instructions:
                si = getattr(ins, "sync_info", None)
                lim = 1 if isinstance(ins, mybir.InstDrain) else MAX_WAITS_PER_INST
                if si is not None and si.on_wait and len(si.on_wait) > lim:
                    waits = list(si.on_wait)
                    keep = waits[:lim]
                    extra = waits[lim:]
                    # A drain with a big wait-set is the kernel-tail barrier:
                    # spread its waits across all engines so they resolve in
                    # parallel (the all-engine barrier right after joins them).
                    spread = (
                        isinstance(ins, mybir.InstDrain) and len(extra) > 4
                    )
                    for i, w in enumerate(extra):
                        eng = (
                            _ALL_ENGINES[i % len(_ALL_ENGINES)]
                            if spread
                            else ins.engine
                        )
                        nop = mybir.InstNoOp(
                            name=nc.get_next_instruction_name(),
                            sync_info=mybir.SyncInfo(on_wait=[w], on_update=[]),
                            bass_nofuse=True,
                            engine=eng,
                        )
                        new_insts.append(nop)
                    si.on_wait = keep
                new_insts.append(ins)
            if len(new_insts) != len(blk.instructions):
                blk.instructions = new_insts


# ---------------------------------------------------------------------------

B, T, C, H = 2, 2048, 2048, 16
D = C // H  # 128
ROPE_BASE = 10000.0
HG = 4  # head groups
HL = H // HG  # heads per core = 4
CL = HL * D  # local width = 512
P = 128
TB = 512  # token block
NTB = T // TB  # 4
KCH = T // P  # 16 k-chunks
QT = T // TB  # 4 q-tiles
SCALE = 1.0 / float(np.sqrt(D))

BF16 = mybir.dt.bfloat16
F32 = mybir.dt.float32
bf16_np = ml_dtypes.bfloat16


def _build_nc():
    nc = bass.Bass(trn_type="TRN2")
    xT = nc.declare_dram_parameter("xT", [C, T], BF16, isOutput=False)
    wqkv = nc.declare_dram_parameter("wqkv", [C, 3 * CL], BF16, isOutput=False)
    wout = nc.declare_dram_parameter("wout", [CL, C], BF16, isOutput=False)
    tabs = nc.declare_dram_parameter("tabs", [2, P, T], BF16, isOutput=False)
    masks = nc.declare_dram_parameter("masks", [P, TB], BF16, isOutput=False)
    out = nc.declare_dram_parameter("out", [T, C], BF16, isOutput=True)

    xT_r = xT[:].rearrange("(ko p) t -> p ko t", p=P)  # [128,16,T]
    wqkv_r = wqkv[:].rearrange("(ko p) m -> p ko m", p=P)  # [128,16,1536]
    wout_r = wout[:].rearrange("(h p) n -> p h n", p=P)  # [128,4,2048]
    out_r = out[:].rearrange("(tc p) n -> tc p n", p=P)  # [16,128,2048]

    with tile.TileContext(nc) as tc:
        consts = tc.alloc_tile_pool(name="consts", bufs=1)
        xcpool = tc.alloc_tile_pool(name="xcpool", bufs=24)
        prepool = tc.alloc_tile_pool(name="prepool", bufs=3)
        swppool = tc.alloc_tile_pool(name="swppool", bufs=4)
        ropepool = tc.alloc_tile_pool(name="ropepool", bufs=3)
        persist = tc.alloc_tile_pool(name="persist", bufs=1)
        ppool = tc.alloc_tile_pool(name="ppool", bufs=8)
        accpool = tc.alloc_tile_pool(name="accpool", bufs=8)
        rpool = tc.alloc_tile_pool(name="rpool", bufs=2)
        outpool = tc.alloc_tile_pool(name="outpool", bufs=3)
        mm_psum = tc.alloc_tile_pool(name="mm_psum", bufs=5, space="PSUM")
        acc_psum = tc.alloc_tile_pool(name="acc_psum", bufs=3, space="PSUM")

        # ---- HAM warmup ----
        # The PE's HAM clock gate starts cold (1.2 GHz, ~3.4us of sustained
        # activity to warm). ~8 dummy matmuls on scratch data cover that
        # window while the framework preamble + first DMAs run.
        warm_sb = prepool.tile([P, TB], BF16, tag="pre")
        nc.gpsimd.memset(warm_sb[:], 1.0)
        warm_ps = mm_psum.tile([P, TB], F32, tag="mm")
        for _ in range(8):
            nc.tensor.matmul(
                warm_ps[:], lhsT=warm_sb[:, :P], rhs=warm_sb[:], start=True, stop=True
            )
        warm_out = prepool.tile([P, TB], BF16, tag="pre")
        nc.scalar.copy(out=warm_out[:], in_=warm_ps[:])

        # ---- input DMAs, ordered by first use ----
        # Per-chunk tiles give per-DMA dependency granularity: the first QKV
        # matmul waits on exactly one x chunk + one w chunk, not the full
        # 8MB. Order: (x0,wq) pairs -> rope tables/mask -> wk -> wv -> x1..3
        # -> wout.
        xc = [[None] * KCH for _ in range(NTB)]
        for kc in range(KCH):
            xc[0][kc] = xcpool.tile([P, TB], BF16, tag="xc", name=f"xc0_{kc}")
            nc.sync.dma_start(out=xc[0][kc][:], in_=xT_r[:, kc, ts(0, TB)])
        w_q = []
        w_k = []
        w_v = []
        for kc in range(KCH):
            w_q.append(consts.tile([P, CL], BF16, name=f"w_q{kc}"))
            nc.sync.dma_start(out=w_q[kc][:], in_=wqkv_r[:, kc, 0:CL])
        cos_sb = consts.tile([P, T], BF16)
        nc.sync.dma_start(out=cos_sb[:], in_=tabs[0])
        sin_sb = consts.tile([P, T], BF16)
        nc.sync.dma_start(out=sin_sb[:], in_=tabs[1])
        mask_sb = consts.tile([P, TB], BF16)
        nc.sync.dma_start(out=mask_sb[:], in_=masks[:])
        for kc in range(KCH):
            w_k.append(consts.tile([P, CL], BF16, name=f"w_k{kc}"))
            nc.sync.dma_start(out=w_k[kc][:], in_=wqkv_r[:, kc, CL : 2 * CL])
        for kc in range(KCH):
            w_v.append(consts.tile([P, CL], BF16, name=f"w_v{kc}"))
            nc.sync.dma_start(out=w_v[kc][:], in_=wqkv_r[:, kc, 2 * CL : 3 * CL])
        ones_sb = consts.tile([P, P], BF16)
        nc.vector.memset(ones_sb[:], 1.0)
        wo_sb = consts.tile([P, HL, C], BF16)  # 2MB, used from phase 3

        # ---- persistent activations ----
        qT_sb = persist.tile([P, HL, T], BF16)  # 2MB
        kT_sb = persist.tile([P, HL, T], BF16)  # 2MB
        v_sb = persist.tile([P, KCH, CL], BF16)  # 2MB
        oT_sb = persist.tile([P, HL, T], BF16)  # 2MB

        # ================= Phase 1: QKV + RoPE =================
        for tb in range(NTB):
            if tb > 0:
                for kc in range(KCH):
                    xc[tb][kc] = xcpool.tile([P, TB], BF16, tag="xc", name=f"xc{tb}_{kc}")
                    nc.sync.dma_start(
                        out=xc[tb][kc][:], in_=xT_r[:, kc, ts(tb, TB)]
                    )
                if tb == 1:
                    nc.sync.dma_start(out=wo_sb[:], in_=wout_r)

            # qT / kT in [D, token] layout + RoPE. Emitted kc-outer in groups
            # of 4 heads: 4 psum chains advance one chunk per DMA arrival, so
            # the PE streams at DMA pace instead of stalling on chunk 15.
            for mg in range(2):
                qps = [mm_psum.tile([P, TB], F32, tag="mm", name=f"qp{mg}_{i}") for i in range(4)]
                for kc in range(KCH):
                    for ml in range(4):
                        m = 4 * mg + ml
                        w_t = w_q[kc] if m < HL else w_k[kc]
                        mc = m if m < HL else m - HL
                        nc.tensor.matmul(
                            qps[ml][:],
                            lhsT=w_t[:, ts(mc, P)],
                            rhs=xc[tb][kc][:],
                            start=(kc == 0),
                            stop=(kc == KCH - 1),
                        )
                for ml in range(4):
                    m = 4 * mg + ml
                    pre = prepool.tile([P, TB], BF16)
                    nc.scalar.copy(out=pre[:], in_=qps[ml][:])
                    # swap partition halves via SBUF->SBUF DMA (DVE cannot
                    # cross partitions). Issued on the Pool queue so the Sync
                    # queue's input-load stream is never head-of-line blocked.
                    h64 = D // 2
                    swp = swppool.tile([P, TB], BF16)
                    nc.gpsimd.dma_start(out=swp[0:h64], in_=pre[h64 : 2 * h64])
                    nc.gpsimd.dma_start(out=swp[h64 : 2 * h64], in_=pre[0:h64])
                    ta = ropepool.tile([P, TB], BF16, tag="ta")
                    tb_ = ropepool.tile([P, TB], BF16, tag="tb")
                    # rope = pre * cosF + swap(pre) * sinS  (sinS = [-sin; +sin])
                    nc.vector.tensor_mul(ta[:], pre[:], cos_sb[:, ts(tb, TB)])
                    nc.vector.tensor_mul(tb_[:], swp[:], sin_sb[:, ts(tb, TB)])
                    dest = (
                        qT_sb[:, m, ts(tb, TB)]
                        if m < HL
                        else kT_sb[:, m - HL, ts(tb, TB)]
                    )
                    nc.vector.tensor_add(dest[:], ta[:], tb_[:])

            # V in natural [token, D] layout
            for tsc in range(TB // P):
                vp = mm_psum.tile([P, TB], F32, tag="mm")
                for kc in range(KCH):
                    nc.tensor.matmul(
                        vp[:],
                        lhsT=xc[tb][kc][:, ts(tsc, P)],
                        rhs=w_v[kc][:],
                        start=(kc == 0),
                        stop=(kc == KCH - 1),
                    )
                nc.vector.tensor_copy(out=v_sb[:, tb * (TB // P) + tsc, :], in_=vp[:])

        # ================= Phase 2+3: attention (qt-outer) + out-proj =====
        add_ctr = [0]

        def tree_engine():
            # 2/3 of the prob-accumulation adds on DVE, 1/3 on the (idle)
            # Pool engine.
            add_ctr[0] += 1
            return nc.gpsimd if add_ctr[0] % 3 == 0 else nc.vector

        def emit_attention(qt, h):
            op = acc_psum.tile([P, TB], F32, tag="acc")
            nkc = (qt + 1) * (TB // P)
            levels = [None] * 6
            for kc in range(nkc):
                # columns q < 128*off are entirely masked for this k-chunk;
                # restrict all work to the valid suffix [qs:TB)
                off = kc - qt * (TB // P)
                qs = max(off, 0) * P
                W = TB - qs
                sp = mm_psum.tile([P, TB], F32, tag="mm")
                nc.tensor.matmul(
                    sp[:, :W],
                    lhsT=kT_sb[:, h, ts(kc, P)],
                    rhs=qT_sb[:, h, qt * TB + qs : (qt + 1) * TB],
                    start=True,
                    stop=True,
                )
                pt = ppool.tile([P, TB], BF16)
                if qs > 0:
                    nc.gpsimd.memset(pt[:, :qs], 0.0)
                nc.scalar.activation(
                    out=pt[:, qs:],
                    in_=sp[:, :W],
                    func=mybir.ActivationFunctionType.Exp,
                    scale=SCALE,
                )
                if off >= 0:
                    # triangular mask touches only the diagonal 128 columns;
                    # the rest of the suffix is fully valid
                    nc.vector.tensor_mul(
                        pt[:, qs : qs + P], pt[:, qs : qs + P], mask_sb[:, :P]
                    )
                nc.tensor.matmul(
                    op[:, qs:],
                    lhsT=v_sb[:, kc, ts(h, P)],
                    rhs=pt[:, qs:],
                    start=(kc == 0),
                    stop=(kc == nkc - 1),
                )
                # binary-counter pairwise tree: sums stay magnitude-balanced,
                # so bf16 accumulation is safe (~0.2% on l)
                carry = pt
                lvl = 0
                while levels[lvl] is not None:
                    s = accpool.tile([P, TB], BF16, name="tsum")
                    tree_engine().tensor_add(s[:], levels[lvl][:], carry[:])
                    levels[lvl] = None
                    carry = s
                    lvl += 1
                levels[lvl] = carry
            # fold leftover levels (nkc=12 leaves an 8-sum and a 4-sum)
            acc = None
            for lvl in range(6):
                if levels[lvl] is None:
                    continue
                if acc is None:
                    acc = levels[lvl]
                else:
                    s = accpool.tile([P, TB], BF16, name="tfold")
                    tree_engine().tensor_add(s[:], levels[lvl][:], acc[:])
                    acc = s
            # l over partitions via one ones-lhsT matmul; r = 1/l on DVE
            lp = acc_psum.tile([P, TB], F32, tag="acc")
            nc.tensor.matmul(lp[:], lhsT=ones_sb[:], rhs=acc[:], start=True, stop=True)
            rt = rpool.tile([P, TB], F32, tag="rt")
            nc.vector.reciprocal(rt[:], lp[:])
            nc.vector.tensor_mul(oT_sb[:, h, ts(qt, TB)], op[:], rt[:])

        ev_ctr = [0]

        def emit_outproj(tcc, tail=False):
            for ncc in range(C // TB):
                outp = mm_psum.tile([P, TB], F32, tag="mm")
                for h in range(HL):
                    nc.tensor.matmul(
                        outp[:],
                        lhsT=oT_sb[:, h, ts(tcc, P)],
                        rhs=wo_sb[:, h, ts(ncc, TB)],
                        start=(h == 0),
                        stop=(h == HL - 1),
                    )
                ot = outpool.tile([P, TB], BF16)
                # spread PSUM evictions: tail flush alternates DVE/ACT (ACT is
                # idle there); mid-phase ACT takes every 3rd (exp-loaded)
                ev_ctr[0] += 1
                use_act = (ev_ctr[0] % 2 == 0) if tail else (ev_ctr[0] % 3 == 0)
                if use_act:
                    nc.scalar.copy(out=ot[:], in_=outp[:])
                else:
                    nc.vector.tensor_copy(out=ot[:], in_=outp[:])
                nc.sync.dma_start(out=out_r[tcc, :, ts(ncc, TB)], in_=ot[:])

        # attention with out-proj delayed one (qt, h) step: by the time the
        # PE stream reaches out-proj for token chunk tcc, the oT writes it
        # needs have had a full head's attention to complete on DVE/ACT.
        pending = []  # token chunks ready for out-proj
        for qt in range(QT):
            for h in range(HL):
                emit_attention(qt, h)
                if pending:
                    emit_outproj(pending.pop(0))
            pending.extend(range(qt * (TB // P), (qt + 1) * (TB // P)))
        for tcc in pending:
            emit_outproj(tcc, tail=True)

        for pool in (
            acc_psum,
            mm_psum,
            outpool,
            rpool,
            accpool,
            ppool,
            persist,
            ropepool,
            swppool,
            prepool,
            xcpool,
            consts,
        ):
            pool.release()

    _split_sync_waits(nc)
    return nc


def _host_inputs(x, W_qkv, W_out):
    """Build per-core input maps. Core j: batch j//HG, head-group j%HG."""
    perm = np.concatenate([np.arange(0, D, 2), np.arange(1, D, 2)])  # deinterleave

    # rope tables in de-interleaved layout: rows [0:64]=even-dim freq, dup below
    inv = 1.0 / (ROPE_BASE ** (np.arange(0, D, 2, dtype=np.float32) / D))  # [64]
    ang = np.arange(T, dtype=np.float32)[None, :] * inv[:, None]  # [64, T]
    cosF = np.concatenate([np.cos(ang), np.cos(ang)], axis=0)  # [128, T]
    sinS = np.concatenate([-np.sin(ang), np.sin(ang)], axis=0)  # sign folded
    tabs = np.stack([cosF, sinS]).astype(bf16_np)  # [2,128,T]

    kk = np.arange(P)[:, None]
    qq = np.arange(TB)[None, :]
    mask = (qq >= kk).astype(np.float32).astype(bf16_np)  # [128,TB]

    in_maps = []
    for j in range(8):
        b, hg = j // HG, j % HG
        xTb = np.ascontiguousarray(x[b].T).astype(bf16_np)  # [C, T]
        cols = []
        for part in range(2):  # q, k with permuted D
            for h in range(HL):
                base = part * C + (hg * HL + h) * D
                cols.append(W_qkv[:, base + perm])
        for h in range(HL):  # v natural
            base = 2 * C + (hg * HL + h) * D
            cols.append(W_qkv[:, base : base + D])
        wq = np.concatenate(cols, axis=1).astype(bf16_np)  # [C, 3*CL]
        wo = W_out[hg * CL : (hg + 1) * CL, :].astype(bf16_np)  # [CL, C]
        in_maps.append({"xT": xTb, "wqkv": wq, "wout": wo, "tabs": tabs, "masks": mask})
    return in_maps


def kernel(x, W_qkv, W_out, _trace=False, _tmpdir=None):
    x = np.asarray(x, dtype=np.float32)
    W_qkv = np.asarray(W_qkv, dtype=np.float32)
    W_out = np.asarray(W_out, dtype=np.float32)

    nc = _build_nc()
    in_maps = _host_inputs(x, W_qkv, W_out)
    res = run_bass_kernel_spmd(
        nc, in_maps, core_ids=list(range(8)), trace=_trace, tmpdir=_tmpdir
    )

    out = np.zeros((B, T, C), dtype=np.float32)
    for j in range(8):
        out[j // HG] += np.asarray(res.results[j]["out"], dtype=np.float32)
    if _trace:
        return out, res
    return out


# revision 11
# speedup vs baseline: 1.0546x; 1.0546x over previous
"""Trainium2 Bass kernel for MultiHeadAttentionWithRope.

Problem: B=2, T=2048, C=2048, H=16 heads, D=128 head_dim, fp32 I/O.
  qkv = x @ W_qkv; q,k -> RoPE (adjacent-pair, torchtune) -> causal SDPA
  -> out = o @ W_out

Sharding (8 cores): 2 batches x 4 head-groups (4 heads each).
Each core computes a partial output out_partial[b] = o_heads @ W_out_rows
(bf16); the host sums the 4 partials per batch in f32.

Per-core layout trick: everything is computed in "transposed" space.
  - host pre-transposes x[b] -> xT [C, T] (bf16)
  - qT,kT = (W_q|k)^T x in [D, T] layout directly (lhsT = W slice as stored)
  - RoPE dims are de-interleaved by permuting W_qkv q/k columns on the host
    (even dims first). S = q.k is invariant under a shared permutation.
  - scores are computed transposed: S.T[k,q] = matmul(lhsT=kT, rhs=qT),
    so softmax probs P~[k,q] feed the PV matmul with no on-device transpose.
  - no max-subtraction in softmax (logits are bounded: |S|<~6 at this scale)
  - row-sums l: probs chunks are pairwise tree-summed in bf16 on DVE/Pool,
    then ONE ones-lhsT matmul per (qt,h) reduces the accumulated [128,512]
    tile over partitions (vs. a rowsum matmul per k-chunk: ~25us less PE).
  - 1/l via DVE reciprocal (frees the ACT engine, which exp saturates);
    1/l folded into oT before the output projection.
All matmuls in bf16 (fp32 accumulate in PSUM).

Startup: per-chunk x/w tiles (fine-grained DMA deps) and QKV matmuls
emitted in groups of 4 heads kc-outer, so the PE consumes chunks at DMA
arrival pace instead of waiting for whole tensors.
"""

import sys

sys.path.insert(0, "/opt/trn_rl_repo")

import numpy as np
import ml_dtypes

import concourse.bass as bass
import concourse.tile as tile
from concourse import mybir
from concourse.bass import ts
from concourse.bass_utils import run_bass_kernel_spmd

# Provide antenv.axon_hooks (absent in this container) so trace=True can use
# the axon NTFF profiling path.
def _ensure_axon_hooks():
    import types

    try:
        from antenv import axon_hooks  # noqa: F401
        return
    except ImportError:
        pass
    import antenv

    mod = types.ModuleType("antenv.axon_hooks")
    mod._hook = None

    def set_axon_ntff_profile_hook(h):
        mod._hook = h

    def get_axon_ntff_profile_hook():
        return mod._hook

    mod.set_axon_ntff_profile_hook = set_axon_ntff_profile_hook
    mod.get_axon_ntff_profile_hook = get_axon_ntff_profile_hook
    sys.modules["antenv.axon_hooks"] = mod
    antenv.axon_hooks = mod
    try:
        from trn_agent_boot.trn_boot import _ntff_profile_via_ctypes

        hook = _ntff_profile_via_ctypes("/opt/axon/libaxon_pjrt.so")
        if hook is not None:
            mod._hook = hook
    except Exception:
        pass


_ensure_axon_hooks()

# ---------------------------------------------------------------------------
# This walrus build supports only ONE sync-wait command per instruction.
# TileContext's sem assignment can attach several waits to one instruction
# (and its exit drain aggregates many). Post-pass: hoist excess waits onto
# same-engine NoOps inserted immediately before the instruction -- the
# engine blocks on each wait in order, so semantics are identical.
MAX_WAITS_PER_INST = 1


_ALL_ENGINES = [
    mybir.EngineType.PE,
    mybir.EngineType.Activation,
    mybir.EngineType.DVE,
    mybir.EngineType.Pool,
    mybir.EngineType.SP,
]


def _split_sync_waits(nc):
    for f in nc.m.functions:
        for blk in f.blocks:
            new_insts = []
            for ins in blk.instructions:
                si = getattr(ins, "sync_info", None)
                lim = 1 if isinstance(ins, mybir.InstDrain) else MAX_WAITS_PER_INST
                if si is not None and si.on_wait and len(si.on_wait) > lim:
                    waits = list(si.on_wait)
                    keep = waits[:lim]
                    extra = waits[lim:]
                    # A drain with a big wait-set is the kernel-tail barrier:
                    # spread its waits across all engines so they resolve in
                    # parallel (the all-engine barrier right after joins them).
                    spread = (
                        isinstance(ins, mybir.InstDrain) and len(extra) > 4
                    )
                    for i, w in enumerate(extra):
                        eng = (
                            _ALL_ENGINES[i % len(_ALL_ENGINES)]
                            if spread
                            else ins.engine
                        )
                        nop = mybir.InstNoOp(
                            name=nc.get_next_instruction_name(),
                            sync_info=mybir.SyncInfo(on_wait=[w], on_update=[]),
                            bass_nofuse=True,
                            engine=eng,
                        )
                        new_insts.append(nop)
                    si.on_wait = keep
                new_insts.append(ins)
            if len(new_insts) != len(blk.instructions):
                blk.instructions = new_insts


# ---------------------------------------------------------------------------

B, T, C, H = 2, 2048, 2048, 16
D = C // H  # 128
ROPE_BASE = 10000.0
HG = 4  # head groups
HL = H // HG  # heads per core = 4
CL = HL * D  # local width = 512
P = 128
TB = 512  # token block
NTB = T // TB  # 4
KCH = T // P  # 16 k-chunks
QT = T // TB  # 4 q-tiles
SCALE = 1.0 / float(np.sqrt(D))

BF16 = mybir.dt.bfloat16
F32 = mybir.dt.float32
bf16_np = ml_dtypes.bfloat16


def _build_nc():
    nc = bass.Bass(trn_type="TRN2")
    xT = nc.declare_dram_parameter("xT", [C, T], BF16, isOutput=False)
    wqkv = nc.declare_dram_parameter("wqkv", [C, 3 * CL], BF16, isOutput=False)
    wout = nc.declare_dram_parameter("wout", [CL, C], BF16, isOutput=False)
    tabs = nc.declare_dram_parameter("tabs", [2, P, T], BF16, isOutput=False)
    masks = nc.declare_dram_parameter("masks", [2, P, P], BF16, isOutput=False)
    out = nc.declare_dram_parameter("out", [T, C], BF16, isOutput=True)

    xT_r = xT[:].rearrange("(ko p) t -> p ko t", p=P)  # [128,16,T]
    wqkv_r = wqkv[:].rearrange("(ko p) m -> p ko m", p=P)  # [128,16,1536]
    wout_r = wout[:].rearrange("(h p) n -> p h n", p=P)  # [128,4,2048]
    out_r = out[:].rearrange("(tc p) n -> tc p n", p=P)  # [16,128,2048]

    with tile.TileContext(nc) as tc:
        consts = tc.alloc_tile_pool(name="consts", bufs=1)
        xcpool = tc.alloc_tile_pool(name="xcpool", bufs=6)
        prepool = tc.alloc_tile_pool(name="prepool", bufs=3)
        swppool = tc.alloc_tile_pool(name="swppool", bufs=4)
        ropepool = tc.alloc_tile_pool(name="ropepool", bufs=3)
        persist = tc.alloc_tile_pool(name="persist", bufs=1)
        ppool = tc.alloc_tile_pool(name="ppool", bufs=8)
        accpool = tc.alloc_tile_pool(name="accpool", bufs=7)
        rpool = tc.alloc_tile_pool(name="rpool", bufs=2)
        outpool = tc.alloc_tile_pool(name="outpool", bufs=3)
        mm_psum = tc.alloc_tile_pool(name="mm_psum", bufs=4, space="PSUM")
        acc_psum = tc.alloc_tile_pool(name="acc_psum", bufs=4, space="PSUM")

        # ---- HAM warmup ----
        # The PE's HAM clock gate starts cold (1.2 GHz, ~3.4us of sustained
        # activity to warm). ~8 dummy matmuls on scratch data cover that
        # window while the framework preamble + first DMAs run.
        warm_sb = prepool.tile([P, TB], BF16, tag="pre")
        nc.gpsimd.memset(warm_sb[:], 1.0)
        warm_ps = mm_psum.tile([P, TB], F32, tag="mm")
        for _ in range(8):
            nc.tensor.matmul(
                warm_ps[:], lhsT=warm_sb[:, :P], rhs=warm_sb[:], start=True, stop=True
            )
        warm_out = prepool.tile([P, TB], BF16, tag="pre")
        nc.scalar.copy(out=warm_out[:], in_=warm_ps[:])

        # ---- input DMAs, ordered by first use ----
        # Per-chunk tiles give per-DMA dependency granularity: the first QKV
        # matmul waits on exactly one x chunk + one w chunk, not the full
        # 8MB. Order: (x0,wq) pairs -> rope tables/mask -> wk -> wv -> x1..3
        # -> wout.
        # x and w_q interleaved in quarters (4 k-chunks per DMA): the first
        # QKV matmul group waits one quarter, and the ~600ns/instr serial DMA
        # issue cost on the Sync queue stays small.
        xq = [[None] * 4 for _ in range(NTB)]
        w_q = []
        for q4 in range(4):
            w_q.append(consts.tile([P, 4, CL], BF16, name=f"w_q{q4}"))
            nc.sync.dma_start(
                out=w_q[q4][:], in_=wqkv_r[:, ts(q4, 4), 0:CL]
            )
            xq[0][q4] = xcpool.tile([P, 4, TB], BF16, tag="xc", name=f"xq0_{q4}")
            nc.sync.dma_start(
                out=xq[0][q4][:], in_=xT_r[:, ts(q4, 4), ts(0, TB)]
            )
        cos_sb = consts.tile([P, T], BF16)
        nc.sync.dma_start(out=cos_sb[:], in_=tabs[0])
        sin_sb = consts.tile([P, T], BF16)
        nc.sync.dma_start(out=sin_sb[:], in_=tabs[1])
        ident_sb = consts.tile([P, P], BF16)
        nc.sync.dma_start(out=ident_sb[:], in_=masks[0])
        tri_sb = consts.tile([P, P], BF16)
        nc.sync.dma_start(out=tri_sb[:], in_=masks[1])
        w_k = []
        w_v = []
        for h2 in range(2):
            w_k.append(consts.tile([P, 8, CL], BF16, name=f"w_k{h2}"))
            nc.sync.dma_start(
                out=w_k[h2][:], in_=wqkv_r[:, ts(h2, 8), CL : 2 * CL]
            )
        for h2 in range(2):
            w_v.append(consts.tile([P, 8, CL], BF16, name=f"w_v{h2}"))
            nc.sync.dma_start(
                out=w_v[h2][:], in_=wqkv_r[:, ts(h2, 8), 2 * CL : 3 * CL]
            )
        ones_sb = consts.tile([P, P], BF16)
        nc.vector.memset(ones_sb[:], 1.0)
        wo_sb = consts.tile([P, HL, C], BF16)  # 2MB, used from phase 3

        # ---- persistent activations ----
        qT_sb = persist.tile([P, HL, T], BF16)  # 2MB
        kT_sb = persist.tile([P, HL, T], BF16)  # 2MB
        v_sb = persist.tile([P, KCH, CL], BF16)  # 2MB
        oT_sb = persist.tile([P, HL, T], BF16)  # 2MB

        # ================= Phase 1: QKV + RoPE =================
        for tb in range(NTB):
            if tb > 0:
                for q4 in range(4):
                    xq[tb][q4] = xcpool.tile(
                        [P, 4, TB], BF16, tag="xc", name=f"xq{tb}_{q4}"
                    )
                    nc.sync.dma_start(
                        out=xq[tb][q4][:], in_=xT_r[:, ts(q4, 4), ts(tb, TB)]
                    )
                if tb == 1:
                    nc.sync.dma_start(out=wo_sb[:], in_=wout_r)

            # qT / kT in [D, token] layout + RoPE. Emitted kc-outer in groups
            # of 4 heads: 4 psum chains advance one chunk per DMA arrival, so
            # the PE streams at DMA pace instead of stalling on chunk 15.
            for mg in range(2):
                qps = [mm_psum.tile([P, TB], F32, tag="mm", name=f"qp{mg}_{i}") for i in range(4)]
                for kc in range(KCH):
                    for ml in range(4):
                        m = 4 * mg + ml
                        if m < HL:
                            w_t = w_q[kc // 4][:, kc % 4, ts(m, P)]
                        else:
                            w_t = w_k[kc // 8][:, kc % 8, ts(m - HL, P)]
                        nc.tensor.matmul(
                            qps[ml][:],
                            lhsT=w_t,
                            rhs=xq[tb][kc // 4][:, kc % 4, :],
                            start=(kc == 0),
                            stop=(kc == KCH - 1),
                        )
                for ml in range(4):
                    m = 4 * mg + ml
                    pre = prepool.tile([P, TB], BF16)
                    nc.scalar.copy(out=pre[:], in_=qps[ml][:])
                    # swap partition halves via SBUF->SBUF DMA (DVE cannot
                    # cross partitions). Issued on the Pool queue so the Sync
                    # queue's input-load stream is never head-of-line blocked.
                    h64 = D // 2
                    swp = swppool.tile([P, TB], BF16)
                    nc.gpsimd.dma_start(out=swp[0:h64], in_=pre[h64 : 2 * h64])
                    nc.gpsimd.dma_start(out=swp[h64 : 2 * h64], in_=pre[0:h64])
                    ta = ropepool.tile([P, TB], BF16, tag="ta")
                    tb_ = ropepool.tile([P, TB], BF16, tag="tb")
                    # rope = pre * cosF + swap(pre) * sinS  (sinS = [-sin; +sin])
                    nc.vector.tensor_mul(ta[:], pre[:], cos_sb[:, ts(tb, TB)])
                    nc.vector.tensor_mul(tb_[:], swp[:], sin_sb[:, ts(tb, TB)])
                    dest = (
                        qT_sb[:, m, ts(tb, TB)]
                        if m < HL
                        else kT_sb[:, m - HL, ts(tb, TB)]
                    )
                    nc.vector.tensor_add(dest[:], ta[:], tb_[:])

            # V in natural [token, D] layout
            for tsc in range(TB // P):
                vp = mm_psum.tile([P, TB], F32, tag="mm")
                for kc in range(KCH):
                    nc.tensor.matmul(
                        vp[:],
                        lhsT=xq[tb][kc // 4][:, kc % 4, ts(tsc, P)],
                        rhs=w_v[kc // 8][:, kc % 8, :],
                        start=(kc == 0),
                        stop=(kc == KCH - 1),
                    )
                nc.vector.tensor_copy(out=v_sb[:, tb * (TB // P) + tsc, :], in_=vp[:])

        # ================= Phase 2+3: attention (qt-outer) + out-proj =====
        add_ctr = [0]

        def tree_engine():
            # L1 prob-sum adds: every other one on the (idle) Pool engine
            add_ctr[0] += 1
            return nc.gpsimd if add_ctr[0] % 2 == 0 else nc.vector

        def emit_attention(qt, h):
            op = acc_psum.tile([P, TB], F32, tag="acc")
            lp = acc_psum.tile([P, TB], F32, tag="acc")
            nkc = (qt + 1) * (TB // P)
            nl2 = nkc // 4
            l2count = 0
            pair = None  # L1-pending prob chunk
            l1 = None  # L2-pending L1 sum
            for kc in range(nkc):
                # columns q < 128*off are entirely masked for this k-chunk;
                # restrict all work to the valid suffix [qs:TB)
                off = kc - qt * (TB // P)
                qs = max(off, 0) * P
                W = TB - qs
                sp = mm_psum.tile([P, TB], F32, tag="mm")
                nc.tensor.matmul(
                    sp[:, :W],
                    lhsT=kT_sb[:, h, ts(kc, P)],
                    rhs=qT_sb[:, h, qt * TB + qs : (qt + 1) * TB],
                    start=True,
                    stop=(off < 0),
                )
                if off >= 0:
                    # causal mask: accumulate a -1e9 upper-triangle bias into
                    # the 128 diagonal columns (one cheap PE matmul instead of
                    # a DVE multiply on the whole chunk)
                    nc.tensor.matmul(
                        sp[:, :P],
                        lhsT=ident_sb[:],
                        rhs=tri_sb[:],
                        start=False,
                        stop=True,
                    )
                pt = ppool.tile([P, TB], BF16)
                if qs > 0:
                    nc.gpsimd.memset(pt[:, :qs], 0.0)
                nc.scalar.activation(
                    out=pt[:, qs:],
                    in_=sp[:, :W],
                    func=mybir.ActivationFunctionType.Exp,
                    scale=SCALE,
                )
                nc.tensor.matmul(
                    op[:, qs:],
                    lhsT=v_sb[:, kc, ts(h, P)],
                    rhs=pt[:, qs:],
                    start=(kc == 0),
                    stop=(kc == nkc - 1),
                )
                # pairwise 4-chunk sums in bf16 (magnitude-balanced, ~0.1% on
                # l); each 4-chunk sum feeds an accumulating ones-lhsT rowsum
                # matmul, so the PE reduces l over partitions at 1/4 the
                # columns of a per-chunk rowsum
                if pair is None:
                    pair = pt
                else:
                    s1 = accpool.tile([P, TB], BF16, name="l1sum")
                    tree_engine().tensor_add(s1[:], pair[:], pt[:])
                    pair = None
                    if l1 is None:
                        l1 = s1
                    else:
                        s2 = accpool.tile([P, TB], BF16, name="l2sum")
                        nc.vector.tensor_add(s2[:], l1[:], s1[:])
                        l1 = None
                        nc.tensor.matmul(
                            lp[:],
                            lhsT=ones_sb[:],
                            rhs=s2[:],
                            start=(l2count == 0),
                            stop=(l2count == nl2 - 1),
                        )
                        l2count += 1
            # r = 1/l = exp(-ln(l)) on ACT (custom-DVE recip ops don't
            # compile on this walrus build; exact DVE reciprocal is 3.3us/tile)
            lt = rpool.tile([P, TB], F32, tag="lt")
            nc.scalar.activation(
                out=lt[:], in_=lp[:], func=mybir.ActivationFunctionType.Ln
            )
            rt = rpool.tile([P, TB], F32, tag="rt")
            nc.scalar.activation(
                out=rt[:],
                in_=lt[:],
                func=mybir.ActivationFunctionType.Exp,
                scale=-1.0,
            )
            nc.vector.tensor_mul(oT_sb[:, h, ts(qt, TB)], op[:], rt[:])

        ev_ctr = [0]

        def emit_outproj(tcc, tail=False):
            for ncc in range(C // TB):
                outp = mm_psum.tile([P, TB], F32, tag="mm")
                for h in range(HL):
                    nc.tensor.matmul(
                        outp[:],
                        lhsT=oT_sb[:, h, ts(tcc, P)],
                        rhs=wo_sb[:, h, ts(ncc, TB)],
                        start=(h == 0),
                        stop=(h == HL - 1),
                    )
                ot = outpool.tile([P, TB], BF16)
                # spread PSUM evictions: tail flush alternates DVE/ACT (ACT is
                # idle there); mid-phase ACT takes every 3rd (exp-loaded)
                ev_ctr[0] += 1
                use_act = (ev_ctr[0] % 2 == 0) if tail else (ev_ctr[0] % 4 == 0)
                if use_act:
                    nc.scalar.copy(out=ot[:], in_=outp[:])
                else:
                    nc.vector.tensor_copy(out=ot[:], in_=outp[:])
                nc.sync.dma_start(out=out_r[tcc, :, ts(ncc, TB)], in_=ot[:])

        # attention with out-proj delayed one (qt, h) step: by the time the
        # PE stream reaches out-proj for token chunk tcc, the oT writes it
        # needs have had a full head's attention to complete on DVE/ACT.
        pending = []  # token chunks ready for out-proj
        for qt in range(QT):
            for h in range(HL):
                emit_attention(qt, h)
                if pending:
                    emit_outproj(pending.pop(0))
            pending.extend(range(qt * (TB // P), (qt + 1) * (TB // P)))
        for tcc in pending:
            emit_outproj(tcc, tail=True)

        for pool in (
            acc_psum,
            mm_psum,
            outpool,
            rpool,
            accpool,
            ppool,
            persist,
            ropepool,
            swppool,
            prepool,
            xcpool,
            consts,
        ):
            pool.release()

    _split_sync_waits(nc)
    return nc


def _host_inputs(x, W_qkv, W_out):
    """Build per-core input maps. Core j: batch j//HG, head-group j%HG."""
    perm = np.concatenate([np.arange(0, D, 2), np.arange(1, D, 2)])  # deinterleave

    # rope tables in de-interleaved layout: rows [0:64]=even-dim freq, dup below
    inv = 1.0 / (ROPE_BASE ** (np.arange(0, D, 2, dtype=np.float32) / D))  # [64]
    ang = np.arange(T, dtype=np.float32)[None, :] * inv[:, None]  # [64, T]
    cosF = np.concatenate([np.cos(ang), np.cos(ang)], axis=0)  # [128, T]
    sinS = np.concatenate([-np.sin(ang), np.sin(ang)], axis=0)  # sign folded
    tabs = np.stack([cosF, sinS]).astype(bf16_np)  # [2,128,T]

    kk = np.arange(P)[:, None]
    qq = np.arange(P)[None, :]
    ident = np.eye(P, dtype=np.float32)
    tri = np.where(qq >= kk, 0.0, -1e9).astype(np.float32)
    mask = np.stack([ident, tri]).astype(bf16_np)  # [2,128,128]

    in_maps = []
    for j in range(8):
        b, hg = j // HG, j % HG
        xTb = np.ascontiguousarray(x[b].T).astype(bf16_np)  # [C, T]
        cols = []
        for part in range(2):  # q, k with permuted D
            for h in range(HL):
                base = part * C + (hg * HL + h) * D
                cols.append(W_qkv[:, base + perm])
        for h in range(HL):  # v natural
            base = 2 * C + (hg * HL + h) * D
            cols.append(W_qkv[:, base : base + D])
        wq = np.concatenate(cols, axis=1).astype(bf16_np)  # [C, 3*CL]
        wo = W_out[hg * CL : (hg + 1) * CL, :].astype(bf16_np)  # [CL, C]
        in_maps.append({"xT": xTb, "wqkv": wq, "wout": wo, "tabs": tabs, "masks": mask})
    return in_maps


def kernel(x, W_qkv, W_out, _trace=False, _tmpdir=None):
    x = np.asarray(x, dtype=np.float32)
    W_qkv = np.asarray(W_qkv, dtype=np.float32)
    W_out = np.asarray(W_out, dtype=np.float32)

    nc = _build_nc()
    in_maps = _host_inputs(x, W_qkv, W_out)
    res = run_bass_kernel_spmd(
        nc, in_maps, core_ids=list(range(8)), trace=_trace, tmpdir=_tmpdir
    )

    out = np.zeros((B, T, C), dtype=np.float32)
    for j in range(8):
        out[j // HG] += np.asarray(res.results[j]["out"], dtype=np.float32)
    if _trace:
        return out, res
    return out
